# revision 2
# baseline (speedup 1.0000x reference)
"""Trainium2 Bass kernel for the 4-chain masked RNN (ArbitraryStructureRNN).

Structure: out = last step of (layer2 + skip), where
  layer0: x -> h0 (RNN tanh, W_ih0 unmasked)
  layer1: h0 -> h1 (masked W_ih1)
  layer2: h1 -> h2 (masked W_ih2)
  skip:   h0 -> hs (masked W_ihs), added to layer2 output at the end.

Sharding: data-parallel over batch (B=64 -> 8 cores x 8).
Per core all 4 chains run as a wavefront (chain c lags its input producer
by one 16-step group). Input projections for a 16-step group are matmul'd
into the PSUM bank first (sets has_written), the serial recurrence matmuls
then accumulate on top (start=False) so the per-step add is free; tanh is
a single strided ACT per chain-step reading PSUM and writing the bf16
h-history in SBUF, which feeds both the next step's matmuls and the
downstream chain's projections.
"""

import sys, types

for _p in ("/opt/trn_rl_repo",):
    if _p not in sys.path:
        sys.path.append(_p)

import numpy as np

# make run_bass_kernel_spmd(trace=True) usable under axon (optional here)
try:
    import antenv

    if not hasattr(antenv, "axon_hooks"):
        _hooks = types.ModuleType("antenv.axon_hooks")
        _h = [None]
        _hooks.set_axon_ntff_profile_hook = lambda h: _h.__setitem__(0, h)
        _hooks.get_axon_ntff_profile_hook = lambda: _h[0]
        sys.modules["antenv.axon_hooks"] = _hooks
        antenv.axon_hooks = _hooks
        try:
            from trn_agent_boot.trn_boot import _ntff_profile_via_ctypes

            _hooks.set_axon_ntff_profile_hook(
                _ntff_profile_via_ctypes("/opt/axon/libaxon_pjrt.so")
            )
        except Exception:
            pass
except Exception:
    pass

from concourse import bacc, tile
from concourse import bass_utils
from concourse.bass import mybir

BF16 = mybir.dt.bfloat16
F32 = mybir.dt.float32

H = 512
IN = 256
NCORES = 8
GS = 16  # steps per psum bank group

# chain input-feature k-tile counts: c0 reads x (256 = 2 tiles), others read h (4)
KIN = [2, 4, 4, 4]
WM_BASE = [0, 1024, 3072, 5120]  # col base of each chain in wmT
WM_COLS = WM_BASE[-1] + 2048


def _tileize(a):
    """[K, M] -> [128, (K/128)*(M/128)*128] with col = (k*nm + m)*128 + j."""
    K, M = a.shape
    nk, nm = K // 128, M // 128
    return (
        a.reshape(nk, 128, nm, 128).transpose(1, 0, 2, 3).reshape(128, nk * nm * 128)
    )


def build(T, b):
    NG = T // GS
    nc = bacc.Bacc("TRN2", target_bir_lowering=False, debug=False, num_devices=NCORES)
    xT_d = nc.dram_tensor("xT", [128, 2 * T * b], BF16, kind="ExternalInput").ap()
    wmT_d = nc.dram_tensor("wmT", [128, WM_COLS], BF16, kind="ExternalInput").ap()
    whT_d = nc.dram_tensor("whT", [128, 8192], BF16, kind="ExternalInput").ap()
    aux_d = nc.dram_tensor("aux", [1, 2560], BF16, kind="ExternalInput").ap()
    bT_d = nc.dram_tensor("biasT", [128, 2048], BF16, kind="ExternalInput").ap()
    out_d = nc.dram_tensor("out", [128, 4 * b], F32, kind="ExternalOutput").ap()

    with tile.TileContext(nc) as tc:
        with (
            tc.tile_pool(name="const", bufs=1) as cpool,
            tc.tile_pool(name="hist", bufs=1) as hpool,
            tc.tile_pool(name="ps", bufs=2, space="PSUM") as ppool,
        ):
            xT = cpool.tile([128, 2 * T * b], BF16, tag="xT")
            wmT = cpool.tile([128, WM_COLS], BF16, tag="wmT")
            whT = cpool.tile([128, 8192], BF16, tag="whT")
            aux = cpool.tile([1, 2560], BF16, tag="aux")
            bT = cpool.tile([128, 2048], BF16, tag="bT")
            nc.sync.dma_start(out=xT[:], in_=xT_d[:])
            nc.sync.dma_start(out=wmT[:], in_=wmT_d[:])
            nc.sync.dma_start(out=whT[:], in_=whT_d[:])
            nc.sync.dma_start(out=aux[:], in_=aux_d[:])
            nc.sync.dma_start(out=bT[:], in_=bT_d[:])

            # h histories (bf16, transposed layout): full for h0/h1 (feed
            # downstream projections), 16-step rings for h2/hs.
            h0 = hpool.tile([128, 4 * T * b], BF16, tag="h0")
            h1 = hpool.tile([128, 4 * T * b], BF16, tag="h1")
            h2 = hpool.tile([128, 4 * GS * b], BF16, tag="h2")
            hs = hpool.tile([128, 4 * GS * b], BF16, tag="hs")
            hist = [h0, h1, h2, hs]
            hlen = [T, T, GS, GS]  # cols per k-tile (in steps)

            out_sb = hpool.tile([128, 4 * b], F32, tag="osb")

            def proj_thunks(c, g, P):
                """Thunk list: proj mms of chain c, steps [GS*g, GS*(g+1)),
                into psum tile P (first mm resets the bank)."""
                t0 = GS * g
                if c == 0:
                    src = xT
                else:
                    src = h0 if c != 2 else h1
                slen = T
                thunks = []
                for m in range(4):
                    for k in range(KIN[c]):
                        def th(m=m, k=k, first=(m == 0 and k == 0)):
                            nc.tensor.matmul(
                                P[:, m * 128 : (m + 1) * 128],
                                wmT[:, WM_BASE[c] + (k * 4 + m) * 128 :][:, :128],
                                src[:, k * slen * b + t0 * b :][:, : GS * b],
                                start=first,
                                stop=False,
                                skip_group_check=True,
                            )
                        thunks.append(th)
                return thunks

            def stt_bias(c, P):
                nc.vector.scalar_tensor_tensor(
                    P[:],
                    P[:],
                    1.0,
                    bT[:, c * 512 : (c + 1) * 512],
                    mybir.AluOpType.mult,
                    mybir.AluOpType.add,
                )

            def recur_act(c, g, j, P):
                """One serial step for chain c: accumulate W_hh @ h_{t-1} into
                P's step slice, then tanh the slice out to the h history."""
                t = GS * g + j
                hbuf = hist[c]
                L = hlen[c]
                if t > 0:
                    tp = (t - 1) % L
                    last = t == T - 1 or j == GS - 1
                    for k in range(4):
                        for m in range(4):
                            nc.tensor.matmul(
                                P[:, m * 128 + j * b :][:, :b],
                                whT[:, c * 2048 + (k * 4 + m) * 128 :][:, :128],
                                hbuf[:, k * L * b + tp * b :][:, :b],
                                start=False,
                                stop=last and k == 3 and m == 3,
                                skip_group_check=True,
                            )
                tc_ = t % L
                pin = P[:].rearrange("p (m t v) -> p m t v", m=4, t=GS)[:, :, j, :]
                hout = hbuf[:].rearrange("p (k t v) -> p k t v", k=4, t=L)[:, :, tc_, :]
                nc.scalar.activation(hout, pin, mybir.ActivationFunctionType.Tanh)

            # wavefront with minimal lags: c0 at group it, c1/skip at it-1,
            # c2 at it-2. Proj mms for iteration it run as a block at the TOP
            # of iteration it (their inputs completed by the end of it-1);
            # the PE stays busy since proj work is on the same engine.
            LAG = {0: 0, 1: 1, 3: 1, 2: 2}

            def grp(c, it):
                g = it - LAG[c]
                return g if 0 <= g < NG else None

            NIT = NG + 2
            cur = [None, None, None, None]

            for it in range(NIT):
                for c in (0, 1, 3, 2):
                    g = grp(c, it)
                    if g is not None:
                        cur[c] = ppool.tile([128, 512], F32, name=f"ps{c}_{it}", tag=f"ps{c}")
                        for th in proj_thunks(c, g, cur[c]):
                            th()
                    else:
                        cur[c] = None
                for c in (0, 1, 3, 2):
                    if cur[c] is not None:
                        stt_bias(c, cur[c])
                for j in range(GS):
                    for c in (0, 1, 3, 2):
                        g = grp(c, it)
                        if g is not None:
                            recur_act(c, g, j, cur[c])

            # out = h2[T-1] + hs[T-1]
            tf = (T - 1) % GS
            h2v = h2[:].rearrange("p (k t v) -> p k t v", k=4, t=GS)[:, :, tf, :]
            hsv = hs[:].rearrange("p (k t v) -> p k t v", k=4, t=GS)[:, :, tf, :]
            ov = out_sb[:].rearrange("p (k v) -> p k v", k=4)
            nc.vector.scalar_tensor_tensor(
                ov, h2v, 1.0, hsv, mybir.AluOpType.mult, mybir.AluOpType.add
            )
            nc.sync.dma_start(out=out_d[:], in_=out_sb[:])
    nc.finalize()
    return nc


def prep_inputs(x, Ws, T, b):
    """Per-core input dicts. Ws = dict of weight arrays (full precision)."""
    wm_list, wh_list, bias_list = [], [], []
    for c, suf in enumerate(["0", "1", "2", "s"]):
        wih = Ws[f"W_ih{suf}"]
        if f"mask{suf}" in Ws:
            wih = wih * Ws[f"mask{suf}"]
        wm_list.append(_tileize(np.ascontiguousarray(wih.T)))
        wh_list.append(_tileize(np.ascontiguousarray(Ws[f"W_hh{suf}"].T)))
        bias_list.append(Ws[f"b_ih{suf}"] + Ws[f"b_hh{suf}"])
    wmT = np.concatenate(wm_list, axis=1).astype(np.float32)
    assert wmT.shape[1] == WM_COLS
    whT = np.concatenate(wh_list, axis=1).astype(np.float32)
    aux = np.zeros((1, 2560), np.float32)
    aux[0, :512] = 1.0
    for c in range(4):
        aux[0, 512 + c * 512 : 512 + (c + 1) * 512] = bias_list[c]

    def bf16(a):
        import ml_dtypes

        return np.asarray(a).astype(ml_dtypes.bfloat16)

    bts = []
    for c in range(4):
        bc = bias_list[c].reshape(4, 128).T.astype(np.float32)  # [128, 4]
        bts.append(np.broadcast_to(bc[:, :, None], (128, 4, 128)).reshape(128, 512))
    biasT = np.concatenate(bts, axis=1)
    wmT, whT, aux, biasT = bf16(wmT), bf16(whT), bf16(aux), bf16(biasT)
    in_maps = []
    for g in range(NCORES):
        xg = x[:T, g * b : (g + 1) * b, :]  # [T, b, IN]
        arr = xg.transpose(2, 0, 1).reshape(IN, T * b)  # [IN, T*b]
        xT = (
            arr.reshape(2, 128, T * b).transpose(1, 0, 2).reshape(128, 2 * T * b)
        ).astype(np.float32)
        in_maps.append(
            {"xT": bf16(xT), "wmT": wmT, "whT": whT, "aux": aux, "biasT": biasT}
        )
    return in_maps


_CACHED = {}


def run(inputs, trace=False):
    inputs = {k: np.asarray(v, np.float32) for k, v in inputs.items()}
    x = np.asarray(inputs["x"], np.float32)
    T, B = x.shape[0], x.shape[1]
    b = B // NCORES
    in_maps = prep_inputs(x, inputs, T, b)
    key = (T, b)
    if key not in _CACHED:
        _CACHED[key] = build(T, b)
    nc = _CACHED[key]
    res = bass_utils.run_bass_kernel_spmd(
        nc, in_maps, core_ids=list(range(NCORES)), trace=trace
    )
    outs = []
    for g in range(NCORES):
        o = res.results[g]["out"]  # [128, 4*b]
        o = o.reshape(128, 4, b).transpose(1, 0, 2).reshape(H, b).T  # [b, H]
        outs.append(o)
    return np.concatenate(outs, axis=0).astype(np.float32), res  # [B, H]


def kernel(**inputs):
    return run(inputs, trace=False)[0]



# revision 4
# speedup vs baseline: 1.3039x; 1.3039x over previous
"""Chain-split pipeline via pair-ReduceScatter: (c0,cs) on even cores,
(c1,c2) on odd cores, b=16 per core.

Per iteration, every core stages [chunkA | chunkB] = [my feed slot | my
chainP h * zsc] and runs a ReduceScatter over pairs [[0,1],[2,3],...].
RS gives rank0 (even) chunkA-sum = its own x window (loopback; odd's
feed is zeros) and rank1 (odd) chunkB-sum = even's h0 (odd's zsc=0
zeroes its own contribution). Both read the SAME output slice -> fully
uniform SPMD program; role differences live in input CONTENT only.

Time shift: chainP slot s processes window s-3 on even cores, s-6 on
odd (delivery pin[it+3] adds 3, producer shift adds 3 more). Warmup
slots produce exact zeros because warmup bias slices are zero (bias
table slice min(s,6), per-core content) and pin/h rings are memset.
Final h2/hs are captured at different slots (double capture); host
picks per type and adds.
"""

import sys, types

for _p in ("/opt/trn_rl_repo",):
    if _p not in sys.path:
        sys.path.append(_p)

import numpy as np

try:
    import antenv

    if not hasattr(antenv, "axon_hooks"):
        _hooks = types.ModuleType("antenv.axon_hooks")
        _h = [None]
        _hooks.set_axon_ntff_profile_hook = lambda h: _h.__setitem__(0, h)
        _hooks.get_axon_ntff_profile_hook = lambda: _h[0]
        sys.modules["antenv.axon_hooks"] = _hooks
        antenv.axon_hooks = _hooks
        try:
            from trn_agent_boot.trn_boot import _ntff_profile_via_ctypes

            _hooks.set_axon_ntff_profile_hook(
                _ntff_profile_via_ctypes("/opt/axon/libaxon_pjrt.so")
            )
        except Exception:
            pass
except Exception:
    pass

from concourse import bacc, tile
from concourse import bass_utils
from concourse.bass import mybir

BF16 = mybir.dt.bfloat16
F32 = mybir.dt.float32

NCORES = 8
GS = 16
B_PER = 16
RING = int(__import__("os").environ.get("V3_RING", "4"))
SKIP = 3   # slots 0..2 fully skipped (state memset to zero)
SA = 3     # even-core shift: slot s = window s-3
SB = 6     # odd-core shift
NBIAS = 7  # bias table slices, index min(slot, 6)
RG = [[0, 1], [2, 3], [4, 5], [6, 7]]


def _tileize(a):
    K, M = a.shape
    nk, nm = K // 128, M // 128
    return a.reshape(nk, 128, nm, 128).transpose(1, 0, 2, 3).reshape(128, nk * nm * 128)


def build(T, b):
    NG = T // GS
    NS = NG + SB          # chainP slots
    NIT = NS + 2          # chainQ trails by 2
    SLOT = 4 * GS * b     # cols per slot, t-major [t, k, v]
    nc = bacc.Bacc("TRN2", target_bir_lowering=False, debug=False, num_devices=NCORES)
    feed_d = nc.dram_tensor("feed", [128, NS * SLOT], BF16, kind="ExternalInput").ap()
    wmP_d = nc.dram_tensor("wmP", [128, 2048], BF16, kind="ExternalInput").ap()
    whP_d = nc.dram_tensor("whP", [128, 2048], BF16, kind="ExternalInput").ap()
    wmQ_d = nc.dram_tensor("wmQ", [128, 2048], BF16, kind="ExternalInput").ap()
    whQ_d = nc.dram_tensor("whQ", [128, 2048], BF16, kind="ExternalInput").ap()
    bP_d = nc.dram_tensor("biasP", [128, NBIAS * SLOT], BF16, kind="ExternalInput").ap()
    bQ_d = nc.dram_tensor("biasQ", [128, NBIAS * SLOT], BF16, kind="ExternalInput").ap()
    zsc_d = nc.dram_tensor("zsc", [128, 1], F32, kind="ExternalInput").ap()
    out_d = nc.dram_tensor("out", [128, 8 * b], F32, kind="ExternalOutput").ap()
    dbg_d = nc.dram_tensor("dbg", [128, 20480], BF16, kind="ExternalOutput").ap()

    with tile.TileContext(nc) as tc:
        with (
            tc.tile_pool(name="const", bufs=1) as cpool,
            tc.tile_pool(name="hist", bufs=1) as hpool,
            tc.tile_pool(name="dram", bufs=1, space="DRAM") as dram,
            tc.tile_pool(name="ps", bufs=2, space="PSUM") as ppool,
        ):
            wmP = cpool.tile([128, 2048], BF16, tag="wmP")
            whP = cpool.tile([128, 2048], BF16, tag="whP")
            wmQ = cpool.tile([128, 2048], BF16, tag="wmQ")
            whQ = cpool.tile([128, 2048], BF16, tag="whQ")
            bP = cpool.tile([128, NBIAS * SLOT], BF16, tag="bP")
            bQ = cpool.tile([128, NBIAS * SLOT], BF16, tag="bQ")
            zsc = cpool.tile([128, 1], F32, tag="zsc")
            for t_, d_ in ((wmP, wmP_d), (whP, whP_d), (wmQ, wmQ_d), (whQ, whQ_d),
                           (bP, bP_d), (bQ, bQ_d), (zsc, zsc_d)):
                nc.sync.dma_start(out=t_[:], in_=d_[:])

            pin = hpool.tile([128, RING * SLOT], BF16, tag="pin")
            hP = hpool.tile([128, RING * SLOT], BF16, tag="hP")
            hQ = hpool.tile([128, 2 * SLOT], BF16, tag="hQ")
            stg = [
                hpool.tile([128, 2 * SLOT], BF16, name=f"stg{i}", tag=f"stg{i}")
                for i in range(2)
            ]
            out_sb = hpool.tile([128, 8 * b], F32, tag="osb")
            nc.vector.memset(pin[:], 0.0)
            nc.vector.memset(hP[:, : SKIP * SLOT], 0.0)
            nc.vector.memset(hQ[:], 0.0)

            # RS chunks are FLAT halves of the input buffer: bin[c] is the
            # chunk delivered to rank c of the pair.
            bins = [dram.tile([2, 128, SLOT], BF16, name=f"bin{i}") for i in range(2)]
            bouts = [dram.tile([128, SLOT], BF16, name=f"bout{i}") for i in range(2)]

            wms = {0: wmP, 1: wmQ}
            whs = {0: whP, 1: whQ}
            bts = {0: bP, 1: bQ}
            hist = {0: hP, 1: hQ}
            nring = {0: RING, 1: 2}

            def proj_thunks(c, s, P):
                """Proj matmuls for chain c slot s into psum pair P=(P01,P23).
                Each tile is a single PSUM bank; cross-bank psum APs are
                silently wrong on this hardware."""
                buf = pin if c == 0 else hP
                src = buf[:].rearrange("p (r t k v) -> p r t k v", r=RING, t=GS, k=4)[
                    :, s % RING
                ]
                thunks = []
                for m in range(4):
                    for k in range(4):
                        def th(m=m, k=k, first=(k == 0)):
                            nc.tensor.matmul(
                                P[m // 2][:, (m % 2) * GS * b : (m % 2 + 1) * GS * b],
                                wms[c][:, (k * 4 + m) * 128 :][:, :128],
                                src[:, :, k, :],
                                start=first and (m % 2 == 0),
                                stop=False,
                                skip_group_check=True,
                            )
                        thunks.append(th)
                return thunks

            def stt_bias(c, s, P):
                u = min(s, NBIAS - 1)
                for h_ in range(2):
                    nc.vector.scalar_tensor_tensor(
                        P[h_][:], P[h_][:], 1.0,
                        bts[c][:, u * SLOT + h_ * 512 : u * SLOT + (h_ + 1) * 512],
                        mybir.AluOpType.mult, mybir.AluOpType.add,
                    )

            def recur_act(c, s, j, P):
                hbuf = hist[c]
                L = nring[c] * GS  # steps held in ring
                t = s * GS + j
                if t > 0:
                    tp = (t - 1) % L
                    last = j == GS - 1
                    for k in range(4):
                        for m in range(4):
                            nc.tensor.matmul(
                                P[m // 2][:, (m % 2) * GS * b + j * b :][:, :b],
                                whs[c][:, (k * 4 + m) * 128 :][:, :128],
                                hbuf[:, (tp * 4 + k) * b :][:, :b],
                                start=False,
                                stop=last and k == 3 and m >= 2,
                                skip_group_check=True,
                            )
                tc_ = t % L
                for h_ in range(2):
                    pin_ap = P[h_][:].rearrange("p (m t v) -> p m t v", m=2, t=GS)[:, :, j, :]
                    hout = hbuf[:].rearrange("p (t k v) -> p t k v", k=4, v=b)[
                        :, tc_, 2 * h_ : 2 * h_ + 2, :
                    ]
                    nc.scalar.activation(hout, pin_ap, mybir.ActivationFunctionType.Tanh)

            def comm(it):
                """Stage + RS delivering pin[it+3] (issued at end of iter it)."""
                s_dst = it + 3
                if not (SKIP <= s_dst < NS):
                    return
                sg = stg[it % 2]
                nc.sync.dma_start(
                    out=sg[:, :SLOT], in_=feed_d[:, s_dst * SLOT : (s_dst + 1) * SLOT]
                )
                hsl = hP[:, (it % RING) * SLOT : (it % RING + 1) * SLOT]
                nc.vector.tensor_scalar(
                    sg[:, SLOT:], hsl, zsc[:, 0:1], None, mybir.AluOpType.mult
                )
                nc.gpsimd.dma_start(out=bins[it % 2][0, :, :], in_=sg[:, :SLOT])
                nc.gpsimd.dma_start(out=bins[it % 2][1, :, :], in_=sg[:, SLOT:])
                nc.gpsimd.collective_compute(
                    "ReduceScatter",
                    mybir.AluOpType.add,
                    replica_groups=RG,
                    ins=[bins[it % 2].opt()],
                    outs=[bouts[it % 2].opt()],
                )
                nc.gpsimd.dma_start(
                    out=pin[:, (s_dst % RING) * SLOT : (s_dst % RING + 1) * SLOT],
                    in_=bouts[it % 2][:],
                )

            # chainP slot = it (skip slots < SKIP); chainQ slot = it - 2
            LAG = {0: 0, 1: 2}

            def grp(c, it):
                s = it - LAG[c]
                return s if SKIP <= s < NS else None

            cur = [None, None]
            nxt = [None, None]

            def build_pending(it):
                pend = []
                for c in (0, 1):
                    s = grp(c, it)
                    if s is not None:
                        nxt[c] = tuple(
                            ppool.tile(
                                [128, 2 * GS * b], F32,
                                name=f"ps{c}_{it}_{h_}", tag=f"ps{c}{h_}",
                            )
                            for h_ in range(2)
                        )
                        pend.extend(proj_thunks(c, s, nxt[c]))
                    else:
                        nxt[c] = None
                return pend

            # comm for the first deliveries (pin[3] at it=0 uses hP[0]=0 memset)
            comm(0)
            comm(1)
            pend = build_pending(2)  # prologue: slot-3 proj runs during it 2
            capA_it = (NG + SA - 1) + LAG[1]  # chainQ slot NG+SA-1 ends here
            capB_it = (NG + SB - 1) + LAG[1]
            for it in range(2, NIT):
                cur, nxt = nxt, [None, None]
                pend2 = build_pending(it + 1) if it + 1 < NIT else []
                for c in range(2):
                    if cur[c] is not None:
                        stt_bias(c, grp(c, it), cur[c])
                for j in range(GS):
                    for c in (0, 1):
                        s = grp(c, it)
                        if s is not None:
                            recur_act(c, s, j, cur[c])
                    lo = (j * len(pend2)) // GS
                    hi = ((j + 1) * len(pend2)) // GS
                    for th in pend2[lo:hi]:
                        th()
                pend = pend2
                comm(it)
                if it == capA_it or it == capB_it:
                    s = grp(1, it)
                    half = 0 if it == capA_it else 1
                    tf = (s * GS + GS - 1) % (2 * GS)
                    hqv = hQ[:].rearrange("p (t k v) -> p t k v", k=4, v=b)[:, tf, :, :]
                    ov = out_sb[:, half * 4 * b : (half + 1) * 4 * b].rearrange(
                        "p (k v) -> p k v", k=4
                    )
                    nc.vector.tensor_scalar_mul(ov, hqv, 1.0)
            nc.sync.dma_start(out=out_d[:], in_=out_sb[:])
            ndbg = min(RING * SLOT, 10240)
            nc.sync.dma_start(out=dbg_d[:, :ndbg], in_=pin[:, :ndbg])
            nc.sync.dma_start(out=dbg_d[:, 10240 : 10240 + ndbg], in_=hP[:, :ndbg])
    nc.finalize()
    return nc


def prep_inputs(x, Ws, T, b):
    import ml_dtypes

    NG = T // GS
    NS = NG + SB
    SLOT = 4 * GS * b

    def bf16(a):
        return np.asarray(a, np.float32).astype(ml_dtypes.bfloat16)

    def wmat(wih, mask=None, pad_in=False):
        w = np.asarray(wih, np.float32)
        if mask is not None:
            w = w * np.asarray(mask, np.float32)
        wT = w.T
        if pad_in:
            full = np.zeros((512, 512), np.float32)
            full[: wT.shape[0]] = wT
            wT = full
        return bf16(_tileize(np.ascontiguousarray(wT)))

    def btile(bih, bhh):
        v = (np.asarray(bih) + np.asarray(bhh)).astype(np.float32)
        m = v.reshape(4, 128).T
        return np.broadcast_to(m[:, :, None], (128, 4, GS * b)).reshape(128, SLOT)

    def btable(bih, bhh, start):
        tb = np.zeros((128, NBIAS * SLOT), np.float32)
        real = btile(bih, bhh)
        for u in range(start, NBIAS):
            tb[:, u * SLOT : (u + 1) * SLOT] = real
        return bf16(tb)

    wA = {
        "wmP": wmat(Ws["W_ih0"], pad_in=True),
        "whP": bf16(_tileize(np.ascontiguousarray(np.asarray(Ws["W_hh0"], np.float32).T))),
        "wmQ": wmat(Ws["W_ihs"], Ws["masks"]),
        "whQ": bf16(_tileize(np.ascontiguousarray(np.asarray(Ws["W_hhs"], np.float32).T))),
        "biasP": btable(Ws["b_ih0"], Ws["b_hh0"], SA),
        "biasQ": btable(Ws["b_ihs"], Ws["b_hhs"], SA),
        "zsc": np.ones((128, 1), np.float32),
    }
    wB = {
        "wmP": wmat(Ws["W_ih1"], Ws["mask1"]),
        "whP": bf16(_tileize(np.ascontiguousarray(np.asarray(Ws["W_hh1"], np.float32).T))),
        "wmQ": wmat(Ws["W_ih2"], Ws["mask2"]),
        "whQ": bf16(_tileize(np.ascontiguousarray(np.asarray(Ws["W_hh2"], np.float32).T))),
        "biasP": btable(Ws["b_ih1"], Ws["b_hh1"], SB),
        "biasQ": btable(Ws["b_ih2"], Ws["b_hh2"], SB),
        "zsc": np.zeros((128, 1), np.float32),
    }
    zfeed = bf16(np.zeros((128, NS * SLOT), np.float32))
    in_maps = []
    for q in range(4):
        xq = np.asarray(x[:T, q * b : (q + 1) * b, :], np.float32)  # [T, b, IN]
        feed = np.zeros((NS, GS, 4, b, 128), np.float32)  # [s, t, k, v, p]
        for g in range(NG):
            blk = xq[g * GS : (g + 1) * GS]  # [GS, b, IN]
            for k in range(2):
                feed[SA + g, :, k] = blk[:, :, k * 128 : (k + 1) * 128].transpose(0, 1, 2)
        feedA = feed.transpose(4, 0, 1, 2, 3).reshape(128, NS * SLOT)
        in_maps.append(dict(wA, feed=bf16(feedA)))
        in_maps.append(dict(wB, feed=zfeed))
    return in_maps


_CACHED = {}


def run(inputs, trace=False, sim=False):
    inputs = {k: np.asarray(v, np.float32) for k, v in inputs.items()}
    x = np.asarray(inputs["x"], np.float32)
    T, B = x.shape[0], x.shape[1]
    b = B // 4
    in_maps = prep_inputs(x, inputs, T, b)
    key = (T, b, "v3")
    if key not in _CACHED:
        _CACHED[key] = build(T, b)
    nc = _CACHED[key]
    if sim:
        from concourse import bass_interp

        bass_interp.pnc_id_to_device_and_real_nc_index = lambda cid: (cid // 8, cid % 8)
        bass_interp.nc_to_real_nc = lambda dev, i: i
        bass_interp.get_device_id_to_routing_id_mapping = lambda: {
            i: i for i in range(16)
        }
        bass_interp.get_dma_mask = lambda a, b: 0xFFFF
        nc.detect_race_conditions = False
        simu = bass_interp.MultiCoreSim(nc, NCORES)
        for i in range(NCORES):
            for k, v in in_maps[i].items():
                simu.cores[i].tensor(k)[:] = v
        simu.simulate()
        outs_raw = [np.asarray(simu.cores[i].mem_tensor("out")) for i in range(NCORES)]
        res = None
    else:
        res = bass_utils.run_bass_kernel_spmd(
            nc, in_maps, core_ids=list(range(NCORES)), trace=trace
        )
        outs_raw = [res.results[i]["out"] for i in range(NCORES)]
    outs = []
    for q in range(4):
        # halves: [0:4b] = capture A (cs), [4b:8b] = capture B (c2)
        a = outs_raw[2 * q][:, : 4 * b].reshape(128, 4, b).transpose(2, 1, 0).reshape(b, 512)
        c2 = (
            outs_raw[2 * q + 1][:, 4 * b :]
            .reshape(128, 4, b)
            .transpose(2, 1, 0)
            .reshape(b, 512)
        )
        outs.append(a + c2)
    return np.concatenate(outs, axis=0).astype(np.float32), res


def kernel(**inputs):
    return run(inputs, trace=False)[0]
